# revision 14
# baseline (speedup 1.0000x reference)
"""2-layer GCN (PyG-style GCNConv) on 8 Trainium2 NeuronCores.

Strategy (v2)
-------------
out = A_hat @ relu(A_hat @ x W1 + b1) @ W2 + b2,  A_hat = D^-1/2 (A+I) D^-1/2.
Aggregate first (A_hat is linear), transform after.

* dinv folding: gather-table rows are pre-scaled by dinv[src]; the aggregated
  PSUM is post-scaled by dinv[dst] at eviction.  Self-loops become ordinary
  edges.  The per-chunk selection matrix is then a pure {0,1} one-hot and is
  stored RESIDENT in SBUF as fp8e4 (exact), loaded once - instead of
  streaming 51MB of bf16 norm matrices from HBM.
* Nodes (padded to 50176) sharded 6272/core; edges partitioned by dst core,
  grouped by (dst-block-of-128, src-table-half) into 128-edge chunks.
  Per chunk one matmul (one-hot lhsT, gathered rows rhs) does scatter+sum.
* Gathers use gpsimd dma_gather with prepare_only+trigger_dma so descriptor
  generation pipelines with the SDMA transfers; calls cover GROUP_BLKS dst
  blocks on rotating SWDGE queues.
* Layer 1 interleaves A/B table chunks in one PSUM accumulation chain.
  Layer 2 runs phase A (table tA, available right after the first
  half-AllGather of h) across all blocks, stashing partial sums in SBUF,
  then phase B once tB lands - overlapping gather work with the collective.
"""

import os
import sys

sys.path.insert(0, "/opt/trn_rl_repo")

import numpy as np
import ml_dtypes

import concourse.bacc as bacc
import concourse.bass as bass
import concourse.mybir as mybir
from concourse.bass_utils import run_bass_kernel_spmd
from concourse.tile import TileContext
from concourse.library_config import mlp

BF16 = mybir.dt.bfloat16
FP32 = mybir.dt.float32
FP8 = mybir.dt.float8e4
I16 = mybir.dt.int16
NPBF16 = ml_dtypes.bfloat16
NPFP8 = ml_dtypes.float8_e4m3

N_CORES = 8
N_RAW = 50000
SHARD = 6272                      # nodes per core (50176 total, padded)
N_PAD = SHARD * N_CORES
NBLK = SHARD // 128               # 49 dst blocks per core
HALF_A = 3200                     # shard rows [0, 3200) -> table A
HALF_B = SHARD - HALF_A           # shard rows [3200, 6272) -> table B
NBLK_A = HALF_A // 128            # 25
NBLK_B = NBLK - NBLK_A            # 24
IN_CH = 256
HID = 128
OUT_CH = 128
GROUP_BLKS = 1                    # dst blocks per gather call

# t2-table slices (within-half block ranges), AllGathered incrementally
SLICES_A = [(0, 9), (9, 8), (17, 8)]
SLICES_B = [(0, 8), (8, 8), (16, 8)]
SROW_A = [0, 9 * 1024, 17 * 1024]
SROW_B = [0, 8 * 1024, 16 * 1024]
ALL_SLICES = SLICES_A + SLICES_B

last_exec_time_ns = None
last_results = None


# ---------------------------------------------------------------- host prep

def _prep(x, edge_index):
    src = np.asarray(edge_index[0], dtype=np.int64)
    dst = np.asarray(edge_index[1], dtype=np.int64)

    deg = np.bincount(dst, minlength=N_PAD).astype(np.float64) + 1.0
    dinv64 = 1.0 / np.sqrt(deg)
    dinv = dinv64.astype(np.float32)

    # self-loops as ordinary edges (for every padded node)
    loop = np.arange(N_PAD, dtype=np.int64)
    src = np.concatenate([src, loop])
    dst = np.concatenate([dst, loop])

    core = dst // SHARD
    blk = (dst % SHARD) // 128
    soff = src % SHARD
    half = (soff >= HALF_A).astype(np.int64)          # 0 = A, 1 = B
    srank = src // SHARD
    dst_off = (dst % 128).astype(np.int64)

    # slice-major table row: row = srow[s] + rank*(128*nb) + (soff%128)*nb
    #                              + (block_in_half - b0)
    bi_half = np.where(half == 0, soff // 128, (soff - HALF_A) // 128)
    srow = np.zeros(src.shape[0], dtype=np.int64)
    nb_arr = np.zeros(src.shape[0], dtype=np.int64)
    b0_arr = np.zeros(src.shape[0], dtype=np.int64)
    for h, slices, srows in ((0, SLICES_A, SROW_A), (1, SLICES_B, SROW_B)):
        for s, (b0, nb) in enumerate(slices):
            m = (half == h) & (bi_half >= b0) & (bi_half < b0 + nb)
            srow[m] = srows[s]
            nb_arr[m] = nb
            b0_arr[m] = b0
    tbl_idx = (srow + srank * (128 * nb_arr) + (soff % 128) * nb_arr
               + (bi_half - b0_arr)).astype(np.int16)

    # chunk counts per (block, half): max over cores (SPMD shared layout)
    gid = core * (2 * NBLK) + half * NBLK + blk
    counts = np.bincount(gid, minlength=N_CORES * 2 * NBLK).reshape(N_CORES, 2, NBLK)
    kA = np.maximum(1, np.ceil(counts[:, 0, :].max(axis=0) / 128).astype(np.int64))
    kB = np.maximum(1, np.ceil(counts[:, 1, :].max(axis=0) / 128).astype(np.int64))

    # global chunk layout: per group g of GROUP_BLKS blocks:
    #   [A-chunks of blocks in g][B-chunks of blocks in g]
    groups = [list(range(g, min(g + GROUP_BLKS, NBLK)))
              for g in range(0, NBLK, GROUP_BLKS)]
    baseA = np.zeros(NBLK, dtype=np.int64)
    baseB = np.zeros(NBLK, dtype=np.int64)
    gArange = []                                       # (chunk0, nchunks) per group
    gBrange = []
    c = 0
    for blks in groups:
        a0 = c
        for b in blks:
            baseA[b] = c
            c += int(kA[b])
        gArange.append((a0, c - a0))
        b0 = c
        for b in blks:
            baseB[b] = c
            c += int(kB[b])
        gBrange.append((b0, c - b0))
    C = c                                              # total chunks per layer
    S = C * 128

    # per-edge slot
    cb = np.where(half == 0, baseA[blk], baseB[blk])
    order = np.lexsort((dst, half, blk, core))
    gsort = gid[order]
    first = np.concatenate([[True], gsort[1:] != gsort[:-1]])
    grp_start = np.flatnonzero(first)
    within = np.arange(order.size) - np.repeat(
        grp_start, np.diff(np.concatenate([grp_start, [order.size]])))
    pos = np.empty_like(order)
    pos[order] = within
    slot = cb * 128 + pos                              # core-local slot id

    # sel: fp8 one-hot, stored K-major: sel[core, k, c, m], slot = c*128 + k
    sel = np.zeros((N_CORES, 128, C, 128), dtype=NPFP8)
    flat = core * (128 * C * 128) + (slot % 128) * (C * 128) + (slot // 128) * 128 + dst_off
    sel.reshape(-1)[flat] = NPFP8(1.0)
    idx16 = np.zeros((N_CORES, S), dtype=np.int16)
    idx16.reshape(-1)[core * S + slot] = tbl_idx

    # wrap idxs: slot j -> partition j%16, col j//16; replicate to 128 partitions
    idx_w = idx16.reshape(N_CORES, S // 16, 16).transpose(0, 2, 1)
    idx_w = np.ascontiguousarray(idx_w)
    idx_w = np.tile(idx_w, (1, 8, 1))                  # [cores, 128, S/16]

    # gather tables: x rows pre-scaled by dinv, K-major within each rank half
    xp = np.zeros((N_PAD, IN_CH), dtype=np.float32)
    xp[:N_RAW] = x
    xp *= dinv[:, None]
    xp = xp.astype(NPBF16)
    xr = xp.reshape(N_CORES, SHARD, IN_CH)

    hA = xr[:, :HALF_A].reshape(N_CORES, NBLK_A, 128, IN_CH)
    hB = xr[:, HALF_A:].reshape(N_CORES, NBLK_B, 128, IN_CH)
    xA = np.ascontiguousarray(np.concatenate(
        [hA[:, b0:b0 + nb].transpose(0, 2, 1, 3).reshape(N_CORES * 128 * nb, IN_CH)
         for (b0, nb) in SLICES_A], axis=0))
    xB = np.ascontiguousarray(np.concatenate(
        [hB[:, b0:b0 + nb].transpose(0, 2, 1, 3).reshape(N_CORES * 128 * nb, IN_CH)
         for (b0, nb) in SLICES_B], axis=0))

    # per-core dinv columns: dinv_col[core][p, b] = dinv[core*SHARD + b*128 + p]
    dinv_col = np.ascontiguousarray(
        dinv.reshape(N_CORES, NBLK, 128).transpose(0, 2, 1))  # [cores, 128, NBLK]

    kAl = [int(v) for v in kA]
    kBl = [int(v) for v in kB]
    return dict(kA=kAl, kB=kBl, baseA=[int(v) for v in baseA],
                baseB=[int(v) for v in baseB], gArange=gArange, gBrange=gBrange,
                groups=groups, C=C, sel=sel, idx_w=idx_w, xA=xA, xB=xB,
                dinv_col=dinv_col)


# ----------------------------------------------------------- device program

def _build(prep):
    kA, kB = prep["kA"], prep["kB"]
    baseA, baseB = prep["baseA"], prep["baseB"]
    gArange, gBrange = prep["gArange"], prep["gBrange"]
    groups = prep["groups"]
    C = prep["C"]
    S = C * 128
    GCH = max(max(n for _, n in gArange), max(n for _, n in gBrange))

    nc = bacc.Bacc("TRN2", target_bir_lowering=False, num_devices=N_CORES,
                   num_swdge_queues=4)

    xA_d = nc.dram_tensor("xA", [N_CORES * HALF_A, IN_CH], BF16, kind="ExternalInput")
    xB_d = nc.dram_tensor("xB", [N_CORES * HALF_B, IN_CH], BF16, kind="ExternalInput")
    sel_d = nc.dram_tensor("sel", [128, C * 128], FP8, kind="ExternalInput")
    idx_d = nc.dram_tensor("idx", [128, S // 16], I16, kind="ExternalInput")
    dinv_d = nc.dram_tensor("dinv", [128, NBLK], FP32, kind="ExternalInput")
    w1_d = nc.dram_tensor("w1", [IN_CH, HID], BF16, kind="ExternalInput")
    w2_d = nc.dram_tensor("w2", [HID, OUT_CH], BF16, kind="ExternalInput")
    b1_d = nc.dram_tensor("b1", [1, HID], BF16, kind="ExternalInput")
    b2_d = nc.dram_tensor("b2", [1, OUT_CH], BF16, kind="ExternalInput")
    ident_d = nc.dram_tensor("ident", [128, 128], BF16, kind="ExternalInput")
    ones_d = nc.dram_tensor("ones", [1, 128], BF16, kind="ExternalInput")
    y_d = nc.dram_tensor("y", [SHARD, OUT_CH], FP32, kind="ExternalOutput")

    bnc2 = [nc.dram_tensor(f"bnc2_{s}", [128, nb * 128], BF16)
            for s, (b0, nb) in enumerate(ALL_SLICES)]
    tA = nc.dram_tensor("tA", [N_CORES * HALF_A, HID], BF16, addr_space="Shared")
    tB = nc.dram_tensor("tB", [N_CORES * HALF_B, HID], BF16, addr_space="Shared")

    def slice_rows(s):
        if s < 3:
            b0, nb = SLICES_A[s]
            return 0, SROW_A[s], 8 * 128 * nb
        b0, nb = SLICES_B[s - 3]
        return 1, SROW_B[s - 3], 8 * 128 * nb

    slice_end = {}
    for s, (b0, nb) in enumerate(ALL_SLICES):
        babs0 = b0 + (0 if s < 3 else NBLK_A)
        slice_end[babs0 + nb - 1] = (s, babs0, nb)

    RG = [list(range(N_CORES))]
    ACT = mybir.ActivationFunctionType

    with TileContext(nc) as tc:
        nc.gpsimd.load_library(mlp)
        import contextlib
        st = contextlib.ExitStack()
        with st:
            consts = st.enter_context(tc.tile_pool(name="consts", bufs=1))
            fpool = st.enter_context(tc.tile_pool(name="fpool", bufs=4))
            ftpool = st.enter_context(tc.tile_pool(name="ftpool", bufs=4))
            opool = st.enter_context(tc.tile_pool(name="opool", bufs=4))
            stash = st.enter_context(tc.tile_pool(name="stash", bufs=NBLK))
            aggps = st.enter_context(tc.tile_pool(name="aggps", bufs=4, space="PSUM"))
            tps = st.enter_context(tc.tile_pool(name="tps", bufs=2, space="PSUM"))
            mmps = st.enter_context(tc.tile_pool(name="mmps", bufs=2, space="PSUM"))

            # ---- constants / resident tensors
            idx_t = consts.tile([128, S // 16], I16)
            nc.sync.dma_start(out=idx_t[:], in_=idx_d[:])
            sel_t = consts.tile([128, C, 128], FP8)
            NSEL = 4
            selsz = [(C // NSEL + (1 if i < C % NSEL else 0)) for i in range(NSEL)]
            off = 0
            for i, sz in enumerate(selsz):
                nc.sync.dma_start(out=sel_t[:, off:off + sz, :],
                                  in_=sel_d[:, off * 128:(off + sz) * 128])
                off += sz
            dinv_t = consts.tile([128, NBLK], FP32)
            nc.sync.dma_start(out=dinv_t[:], in_=dinv_d[:])
            w1_t = consts.tile([128, 2, HID], BF16)
            nc.sync.dma_start(out=w1_t[:], in_=w1_d.rearrange("(c k) m -> k c m", k=128))
            w2_t = consts.tile([128, OUT_CH], BF16)
            nc.sync.dma_start(out=w2_t[:], in_=w2_d[:])
            b1_t = consts.tile([1, HID], BF16)
            nc.sync.dma_start(out=b1_t[:], in_=b1_d[:])
            b2_t = consts.tile([1, OUT_CH], BF16)
            nc.sync.dma_start(out=b2_t[:], in_=b2_d[:])
            ones_t = consts.tile([1, 128], BF16)
            nc.sync.dma_start(out=ones_t[:], in_=ones_d[:])
            ident_t = consts.tile([128, 128], BF16)
            nc.sync.dma_start(out=ident_t[:], in_=ident_d[:])
            h2_t = consts.tile([128, NBLK, HID], BF16)

            qctr = [0]

            def gather(pool, tbl, tbl_ch, c0, n, layer_tag):
                """dma_gather of chunks [c0, c0+n) on a rotating queue."""
                assert n * 128 <= 2048, n
                q = qctr[0] % 4
                qctr[0] += 1
                g = pool.tile([128, GCH, tbl_ch], BF16, tag="g",
                              name=f"g{layer_tag}_{c0}")
                nc.gpsimd.dma_gather(
                    g[:, :n, :], tbl[:], idx_t[:, c0 * 8:(c0 + n) * 8],
                    n * 128, n * 128, tbl_ch, queue_num=q,
                    single_packet=(n * 128 <= 1024))
                return g

            def transform(b, aggf, Wt, nW, bias_t, out_cb):
                mp = mmps.tile([128, 128], FP32, tag="mmps", name=f"mm{id(aggf)}_{b}")
                for kc in range(nW):
                    tp = tps.tile([128, 128], BF16, tag="tp", name=f"tp{id(aggf)}_{b}_{kc}")
                    nc.tensor.transpose(tp[:], aggf[:, kc * 128:(kc + 1) * 128],
                                        ident_t[:])
                    ft = ftpool.tile([128, 128], BF16, tag="fT", name=f"fT{id(aggf)}_{b}_{kc}")
                    nc.scalar.activation(ft[:], tp[:], ACT.Copy)
                    nc.tensor.matmul(mp[:], ft[:], Wt(kc), start=(kc == 0), stop=False)
                nc.tensor.matmul(mp[:], ones_t[:], bias_t[:], start=False, stop=True)
                out_cb(b, mp)

            # ================= layer 1: single phase, A/B interleaved =======
            def l1_out(b, mp):
                # h2 table row = relu(z) * dinv[dst]  (== relu(z*dinv), dinv>0)
                nc.scalar.activation(h2_t[:, b, :], mp[:], ACT.Relu,
                                     scale=dinv_t[:, b:b + 1])
                if b in slice_end:
                    s, babs0, nb = slice_end[b]
                    nc.scalar.dma_start(out=bnc2[s][:],
                                        in_=h2_t[:, babs0:babs0 + nb, :])
                    half, r0, nr = slice_rows(s)
                    tbl = tA if half == 0 else tB
                    nc.gpsimd.collective_compute(
                        "AllGather", mybir.AluOpType.bypass, replica_groups=RG,
                        ins=[bnc2[s][:]], outs=[tbl[r0:r0 + nr, :]])

            with tc.tile_pool(name="gpool1", bufs=8) as gpool1:
                for gi, blks in enumerate(groups):
                    a0, na = gArange[gi]
                    b0, nb = gBrange[gi]
                    gA = gather(gpool1, xA_d, IN_CH, a0, na, "1a")
                    gB = gather(gpool1, xB_d, IN_CH, b0, nb, "1b")
                    for b in blks:
                        ps = aggps.tile([128, IN_CH], FP32, tag="aggps",
                                        name=f"ps1_{b}")
                        for j in range(kA[b]):
                            cg = baseA[b] + j
                            nc.tensor.matmul(ps[:], sel_t[:, cg, :],
                                             gA[:, cg - a0, :], start=(j == 0),
                                             stop=False)
                        for j in range(kB[b]):
                            cg = baseB[b] + j
                            nc.tensor.matmul(ps[:], sel_t[:, cg, :],
                                             gB[:, cg - b0, :], start=False,
                                             stop=(j == kB[b] - 1))
                        aggf = fpool.tile([128, IN_CH], BF16, tag="aggf",
                                          name=f"aggf1_{b}")
                        nc.vector.tensor_scalar_mul(aggf[:], ps[:],
                                                    dinv_t[:, b:b + 1])
                        transform(b, aggf, lambda kc: w1_t[:, kc, :], 2, b1_t,
                                  l1_out)

            # ================= layer 2: phase A (tA), then phase B (tB) =====
            def l2_out(b, mp):
                o = opool.tile([128, OUT_CH], FP32, tag="o", name=f"y{b}")
                nc.scalar.activation(o[:], mp[:], ACT.Copy)
                nc.sync.dma_start(out=y_d[b * 128:(b + 1) * 128, :], in_=o[:])

            aggA = {}
            with tc.tile_pool(name="gpool2", bufs=16) as gpool2:
                groups_eff = groups
                for gi, blks in enumerate(groups_eff):
                    a0, na = gArange[gi]
                    gA = gather(gpool2, tA, HID, a0, na, "2a")
                    for b in blks:
                        ps = aggps.tile([128, HID], FP32, tag="aggps",
                                        name=f"ps2a_{b}")
                        for j in range(kA[b]):
                            cg = baseA[b] + j
                            nc.tensor.matmul(ps[:], sel_t[:, cg, :],
                                             gA[:, cg - a0, :], start=(j == 0),
                                             stop=(j == kA[b] - 1))
                        a = stash.tile([128, HID], BF16, tag="aggA",
                                       name=f"aggA_{b}")
                        aggA[b] = a
                        nc.scalar.activation(a[:], ps[:], ACT.Copy)
                for gi, blks in enumerate(groups_eff):
                    b0, nb = gBrange[gi]
                    gB = gather(gpool2, tB, HID, b0, nb, "2b")
                    for b in blks:
                        ps = aggps.tile([128, HID], FP32, tag="aggps",
                                        name=f"ps2b_{b}")
                        nc.tensor.matmul(ps[:], ident_t[:], aggA[b][:],
                                         start=True, stop=False)
                        for j in range(kB[b]):
                            cg = baseB[b] + j
                            nc.tensor.matmul(ps[:], sel_t[:, cg, :],
                                             gB[:, cg - b0, :], start=False,
                                             stop=(j == kB[b] - 1))
                        aggf = fpool.tile([128, IN_CH], BF16, tag="aggf",
                                          name=f"aggf2_{b}")
                        nc.vector.tensor_scalar_mul(aggf[:, :HID], ps[:],
                                                    dinv_t[:, b:b + 1])
                        transform(b, aggf, lambda kc: w2_t[:], 1, b2_t,
                                  l2_out)

    nc.compile()
    return nc


# ------------------------------------------------------------------- kernel

def kernel(x, edge_index, W1, b1, W2, b2):
    global last_exec_time_ns, last_results
    x = np.asarray(x, dtype=np.float32)
    prep = _prep(x, np.asarray(edge_index))
    nc = _build(prep)

    w1b = np.asarray(W1, dtype=np.float32).astype(NPBF16)
    w2b = np.asarray(W2, dtype=np.float32).astype(NPBF16)
    b1b = np.asarray(b1, dtype=np.float32).reshape(1, -1).astype(NPBF16)
    b2b = np.asarray(b2, dtype=np.float32).reshape(1, -1).astype(NPBF16)
    ident = np.zeros((128, 128), dtype=NPBF16)
    ident[np.arange(128), np.arange(128)] = 1.0

    in_maps = []
    for c in range(N_CORES):
        in_maps.append({
            "xA": prep["xA"], "xB": prep["xB"],
            "sel": prep["sel"][c].reshape(128, -1), "idx": prep["idx_w"][c],
            "dinv": prep["dinv_col"][c],
            "w1": w1b, "w2": w2b, "b1": b1b, "b2": b2b, "ident": ident,
            "ones": np.ones((1, 128), dtype=NPBF16),
        })

    trace = bool(int(os.environ.get("GCN_TRACE", "0")))
    if trace:
        try:
            import ntff_shim
            ntff_shim.install()
        except Exception:
            trace = False
    res = run_bass_kernel_spmd(nc, in_maps, list(range(N_CORES)), trace=trace)
    last_exec_time_ns = res.exec_time_ns
    last_results = res

    y = np.concatenate([np.asarray(res.results[c]["y"]) for c in range(N_CORES)], axis=0)
    return np.ascontiguousarray(y[:N_RAW]).astype(np.float32)


# revision 15
# speedup vs baseline: 1.0426x; 1.0426x over previous
"""2-layer GCN (PyG-style GCNConv) on 8 Trainium2 NeuronCores.

Strategy (v2)
-------------
out = A_hat @ relu(A_hat @ x W1 + b1) @ W2 + b2,  A_hat = D^-1/2 (A+I) D^-1/2.
Aggregate first (A_hat is linear), transform after.

* dinv folding: gather-table rows are pre-scaled by dinv[src]; the aggregated
  PSUM is post-scaled by dinv[dst] at eviction.  Self-loops become ordinary
  edges.  The per-chunk selection matrix is then a pure {0,1} one-hot and is
  stored RESIDENT in SBUF as fp8e4 (exact), loaded once - instead of
  streaming 51MB of bf16 norm matrices from HBM.
* Nodes (padded to 50176) sharded 6272/core; edges partitioned by dst core,
  grouped by (dst-block-of-128, src-table-half) into 128-edge chunks.
  Per chunk one matmul (one-hot lhsT, gathered rows rhs) does scatter+sum.
* Gathers use gpsimd dma_gather with prepare_only+trigger_dma so descriptor
  generation pipelines with the SDMA transfers; calls cover GROUP_BLKS dst
  blocks on rotating SWDGE queues.
* Layer 1 interleaves A/B table chunks in one PSUM accumulation chain.
  Layer 2 runs phase A (table tA, available right after the first
  half-AllGather of h) across all blocks, stashing partial sums in SBUF,
  then phase B once tB lands - overlapping gather work with the collective.
"""

import os
import sys

sys.path.insert(0, "/opt/trn_rl_repo")

import numpy as np
import ml_dtypes

import concourse.bacc as bacc
import concourse.bass as bass
import concourse.mybir as mybir
from concourse.bass_utils import run_bass_kernel_spmd
from concourse.tile import TileContext
from concourse.library_config import mlp

BF16 = mybir.dt.bfloat16
FP32 = mybir.dt.float32
FP8 = mybir.dt.float8e4
I16 = mybir.dt.int16
NPBF16 = ml_dtypes.bfloat16
NPFP8 = ml_dtypes.float8_e4m3

N_CORES = 8
N_RAW = 50000
SHARD = 6272                      # nodes per core (50176 total, padded)
N_PAD = SHARD * N_CORES
NBLK = SHARD // 128               # 49 dst blocks per core
HALF_A = 3200                     # shard rows [0, 3200) -> table A
HALF_B = SHARD - HALF_A           # shard rows [3200, 6272) -> table B
NBLK_A = HALF_A // 128            # 25
NBLK_B = NBLK - NBLK_A            # 24
IN_CH = 256
HID = 128
OUT_CH = 128
GROUP_BLKS = 1                    # dst blocks per gather call

# t2-table slices (within-half block ranges), AllGathered incrementally
SLICES_A = [(0, 9), (9, 8), (17, 8)]
SLICES_B = [(0, 8), (8, 8), (16, 8)]
SROW_A = [0, 9 * 1024, 17 * 1024]
SROW_B = [0, 8 * 1024, 16 * 1024]
ALL_SLICES = SLICES_A + SLICES_B

last_exec_time_ns = None
last_results = None


# ---------------------------------------------------------------- host prep

def _prep(x, edge_index):
    src = np.asarray(edge_index[0], dtype=np.int64)
    dst = np.asarray(edge_index[1], dtype=np.int64)

    deg = np.bincount(dst, minlength=N_PAD).astype(np.float64) + 1.0
    dinv64 = 1.0 / np.sqrt(deg)
    dinv = dinv64.astype(np.float32)

    # self-loops as ordinary edges (for every padded node)
    loop = np.arange(N_PAD, dtype=np.int64)
    src = np.concatenate([src, loop])
    dst = np.concatenate([dst, loop])

    core = dst // SHARD
    blk = (dst % SHARD) // 128
    soff = src % SHARD
    half = (soff >= HALF_A).astype(np.int64)          # 0 = A, 1 = B
    srank = src // SHARD
    dst_off = (dst % 128).astype(np.int64)

    # slice-major table row: row = srow[s] + rank*(128*nb) + (soff%128)*nb
    #                              + (block_in_half - b0)
    bi_half = np.where(half == 0, soff // 128, (soff - HALF_A) // 128)
    srow = np.zeros(src.shape[0], dtype=np.int64)
    nb_arr = np.zeros(src.shape[0], dtype=np.int64)
    b0_arr = np.zeros(src.shape[0], dtype=np.int64)
    for h, slices, srows in ((0, SLICES_A, SROW_A), (1, SLICES_B, SROW_B)):
        for s, (b0, nb) in enumerate(slices):
            m = (half == h) & (bi_half >= b0) & (bi_half < b0 + nb)
            srow[m] = srows[s]
            nb_arr[m] = nb
            b0_arr[m] = b0
    tbl_idx = (srow + srank * (128 * nb_arr) + (soff % 128) * nb_arr
               + (bi_half - b0_arr)).astype(np.int16)

    # chunk counts per (block, half): max over cores (SPMD shared layout)
    gid = core * (2 * NBLK) + half * NBLK + blk
    counts = np.bincount(gid, minlength=N_CORES * 2 * NBLK).reshape(N_CORES, 2, NBLK)
    kA = np.maximum(1, np.ceil(counts[:, 0, :].max(axis=0) / 128).astype(np.int64))
    kB = np.maximum(1, np.ceil(counts[:, 1, :].max(axis=0) / 128).astype(np.int64))

    # global chunk layout: per group g of GROUP_BLKS blocks:
    #   [A-chunks of blocks in g][B-chunks of blocks in g]
    groups = [list(range(g, min(g + GROUP_BLKS, NBLK)))
              for g in range(0, NBLK, GROUP_BLKS)]
    baseA = np.zeros(NBLK, dtype=np.int64)
    baseB = np.zeros(NBLK, dtype=np.int64)
    gArange = []                                       # (chunk0, nchunks) per group
    gBrange = []
    c = 0
    for blks in groups:
        a0 = c
        for b in blks:
            baseA[b] = c
            c += int(kA[b])
        gArange.append((a0, c - a0))
        b0 = c
        for b in blks:
            baseB[b] = c
            c += int(kB[b])
        gBrange.append((b0, c - b0))
    C = c                                              # total chunks per layer
    S = C * 128

    # per-edge slot
    cb = np.where(half == 0, baseA[blk], baseB[blk])
    order = np.lexsort((dst, half, blk, core))
    gsort = gid[order]
    first = np.concatenate([[True], gsort[1:] != gsort[:-1]])
    grp_start = np.flatnonzero(first)
    within = np.arange(order.size) - np.repeat(
        grp_start, np.diff(np.concatenate([grp_start, [order.size]])))
    pos = np.empty_like(order)
    pos[order] = within
    slot = cb * 128 + pos                              # core-local slot id

    # sel: fp8 one-hot, stored K-major: sel[core, k, c, m], slot = c*128 + k
    sel = np.zeros((N_CORES, 128, C, 128), dtype=NPFP8)
    flat = core * (128 * C * 128) + (slot % 128) * (C * 128) + (slot // 128) * 128 + dst_off
    sel.reshape(-1)[flat] = NPFP8(1.0)
    idx16 = np.zeros((N_CORES, S), dtype=np.int16)
    idx16.reshape(-1)[core * S + slot] = tbl_idx

    # wrap idxs: slot j -> partition j%16, col j//16; replicate to 128 partitions
    idx_w = idx16.reshape(N_CORES, S // 16, 16).transpose(0, 2, 1)
    idx_w = np.ascontiguousarray(idx_w)
    idx_w = np.tile(idx_w, (1, 8, 1))                  # [cores, 128, S/16]

    # gather tables: x rows pre-scaled by dinv, K-major within each rank half
    xp = np.zeros((N_PAD, IN_CH), dtype=np.float32)
    xp[:N_RAW] = x
    xp *= dinv[:, None]
    xp = xp.astype(NPBF16)
    xr = xp.reshape(N_CORES, SHARD, IN_CH)

    hA = xr[:, :HALF_A].reshape(N_CORES, NBLK_A, 128, IN_CH)
    hB = xr[:, HALF_A:].reshape(N_CORES, NBLK_B, 128, IN_CH)
    xA = np.ascontiguousarray(np.concatenate(
        [hA[:, b0:b0 + nb].transpose(0, 2, 1, 3).reshape(N_CORES * 128 * nb, IN_CH)
         for (b0, nb) in SLICES_A], axis=0))
    xB = np.ascontiguousarray(np.concatenate(
        [hB[:, b0:b0 + nb].transpose(0, 2, 1, 3).reshape(N_CORES * 128 * nb, IN_CH)
         for (b0, nb) in SLICES_B], axis=0))

    # per-core dinv columns: dinv_col[core][p, b] = dinv[core*SHARD + b*128 + p]
    dinv_col = np.ascontiguousarray(
        dinv.reshape(N_CORES, NBLK, 128).transpose(0, 2, 1))  # [cores, 128, NBLK]

    kAl = [int(v) for v in kA]
    kBl = [int(v) for v in kB]
    return dict(kA=kAl, kB=kBl, baseA=[int(v) for v in baseA],
                baseB=[int(v) for v in baseB], gArange=gArange, gBrange=gBrange,
                groups=groups, C=C, sel=sel, idx_w=idx_w, xA=xA, xB=xB,
                dinv_col=dinv_col)


# ----------------------------------------------------------- device program

def _build(prep):
    kA, kB = prep["kA"], prep["kB"]
    baseA, baseB = prep["baseA"], prep["baseB"]
    gArange, gBrange = prep["gArange"], prep["gBrange"]
    groups = prep["groups"]
    C = prep["C"]
    S = C * 128
    GCH = max(max(n for _, n in gArange), max(n for _, n in gBrange))

    nc = bacc.Bacc("TRN2", target_bir_lowering=False, num_devices=N_CORES,
                   num_swdge_queues=4)

    xA_d = nc.dram_tensor("xA", [N_CORES * HALF_A, IN_CH], BF16, kind="ExternalInput")
    xB_d = nc.dram_tensor("xB", [N_CORES * HALF_B, IN_CH], BF16, kind="ExternalInput")
    sel_d = nc.dram_tensor("sel", [128, C * 128], FP8, kind="ExternalInput")
    idx_d = nc.dram_tensor("idx", [128, S // 16], I16, kind="ExternalInput")
    dinv_d = nc.dram_tensor("dinv", [128, NBLK], FP32, kind="ExternalInput")
    w1_d = nc.dram_tensor("w1", [IN_CH, HID], BF16, kind="ExternalInput")
    w2_d = nc.dram_tensor("w2", [HID, OUT_CH], BF16, kind="ExternalInput")
    b1_d = nc.dram_tensor("b1", [1, HID], BF16, kind="ExternalInput")
    b2_d = nc.dram_tensor("b2", [1, OUT_CH], BF16, kind="ExternalInput")
    ident_d = nc.dram_tensor("ident", [128, 128], BF16, kind="ExternalInput")
    ones_d = nc.dram_tensor("ones", [1, 128], BF16, kind="ExternalInput")
    y_d = nc.dram_tensor("y", [SHARD, OUT_CH], FP32, kind="ExternalOutput")

    bnc2 = [nc.dram_tensor(f"bnc2_{s}", [128, nb * 128], BF16)
            for s, (b0, nb) in enumerate(ALL_SLICES)]
    tA = nc.dram_tensor("tA", [N_CORES * HALF_A, HID], BF16, addr_space="Shared")
    tB = nc.dram_tensor("tB", [N_CORES * HALF_B, HID], BF16, addr_space="Shared")

    def slice_rows(s):
        if s < 3:
            b0, nb = SLICES_A[s]
            return 0, SROW_A[s], 8 * 128 * nb
        b0, nb = SLICES_B[s - 3]
        return 1, SROW_B[s - 3], 8 * 128 * nb

    slice_end = {}
    for s, (b0, nb) in enumerate(ALL_SLICES):
        babs0 = b0 + (0 if s < 3 else NBLK_A)
        slice_end[babs0 + nb - 1] = (s, babs0, nb)

    RG = [list(range(N_CORES))]
    ACT = mybir.ActivationFunctionType

    with TileContext(nc) as tc:
        nc.gpsimd.load_library(mlp)
        import contextlib
        st = contextlib.ExitStack()
        with st:
            consts = st.enter_context(tc.tile_pool(name="consts", bufs=1))
            fpool = st.enter_context(tc.tile_pool(name="fpool", bufs=4))
            ftpool = st.enter_context(tc.tile_pool(name="ftpool", bufs=4))
            opool = st.enter_context(tc.tile_pool(name="opool", bufs=4))
            stash = st.enter_context(tc.tile_pool(name="stash", bufs=NBLK))
            aggps = st.enter_context(tc.tile_pool(name="aggps", bufs=4, space="PSUM"))
            tps = st.enter_context(tc.tile_pool(name="tps", bufs=2, space="PSUM"))
            mmps = st.enter_context(tc.tile_pool(name="mmps", bufs=2, space="PSUM"))

            # ---- constants / resident tensors
            idx_t = consts.tile([128, S // 16], I16)
            nc.sync.dma_start(out=idx_t[:], in_=idx_d[:])
            sel_t = consts.tile([128, C, 128], FP8)
            NSEL = 4
            selsz = [(C // NSEL + (1 if i < C % NSEL else 0)) for i in range(NSEL)]
            off = 0
            for i, sz in enumerate(selsz):
                nc.sync.dma_start(out=sel_t[:, off:off + sz, :],
                                  in_=sel_d[:, off * 128:(off + sz) * 128])
                off += sz
            dinv_t = consts.tile([128, NBLK], FP32)
            nc.sync.dma_start(out=dinv_t[:], in_=dinv_d[:])
            w1_t = consts.tile([128, 2, HID], BF16)
            nc.sync.dma_start(out=w1_t[:], in_=w1_d.rearrange("(c k) m -> k c m", k=128))
            w2_t = consts.tile([128, OUT_CH], BF16)
            nc.sync.dma_start(out=w2_t[:], in_=w2_d[:])
            b1_t = consts.tile([1, HID], BF16)
            nc.sync.dma_start(out=b1_t[:], in_=b1_d[:])
            b2_t = consts.tile([1, OUT_CH], BF16)
            nc.sync.dma_start(out=b2_t[:], in_=b2_d[:])
            ones_t = consts.tile([1, 128], BF16)
            nc.sync.dma_start(out=ones_t[:], in_=ones_d[:])
            ident_t = consts.tile([128, 128], BF16)
            nc.sync.dma_start(out=ident_t[:], in_=ident_d[:])
            h2_t = consts.tile([128, NBLK, HID], BF16)

            qctr = [0]

            def gather(pool, tbl, tbl_ch, c0, n, layer_tag):
                """dma_gather of chunks [c0, c0+n) on a rotating queue."""
                assert n * 128 <= 2048, n
                q = qctr[0] % 4
                qctr[0] += 1
                g = pool.tile([128, GCH, tbl_ch], BF16, tag="g",
                              name=f"g{layer_tag}_{c0}")
                nc.gpsimd.dma_gather(
                    g[:, :n, :], tbl[:], idx_t[:, c0 * 8:(c0 + n) * 8],
                    n * 128, n * 128, tbl_ch, queue_num=q,
                    single_packet=(n * 128 <= 1024))
                return g

            def transform(b, aggf, Wt, nW, bias_t, out_cb):
                mp = mmps.tile([128, 128], FP32, tag="mmps", name=f"mm{id(aggf)}_{b}")
                for kc in range(nW):
                    tp = tps.tile([128, 128], BF16, tag="tp", name=f"tp{id(aggf)}_{b}_{kc}")
                    nc.tensor.transpose(tp[:], aggf[:, kc * 128:(kc + 1) * 128],
                                        ident_t[:])
                    ft = ftpool.tile([128, 128], BF16, tag="fT", name=f"fT{id(aggf)}_{b}_{kc}")
                    nc.scalar.activation(ft[:], tp[:], ACT.Copy)
                    nc.tensor.matmul(mp[:], ft[:], Wt(kc), start=(kc == 0), stop=False)
                nc.tensor.matmul(mp[:], ones_t[:], bias_t[:], start=False, stop=True)
                out_cb(b, mp)

            # ================= layer 1: single phase, A/B interleaved =======
            def l1_out(b, mp):
                # h2 table row = relu(z) * dinv[dst]  (== relu(z*dinv), dinv>0)
                nc.scalar.activation(h2_t[:, b, :], mp[:], ACT.Relu,
                                     scale=dinv_t[:, b:b + 1])
                if b in slice_end:
                    s, babs0, nb = slice_end[b]
                    nc.scalar.dma_start(out=bnc2[s][:],
                                        in_=h2_t[:, babs0:babs0 + nb, :])
                    half, r0, nr = slice_rows(s)
                    tbl = tA if half == 0 else tB
                    nc.gpsimd.collective_compute(
                        "AllGather", mybir.AluOpType.bypass, replica_groups=RG,
                        ins=[bnc2[s][:]], outs=[tbl[r0:r0 + nr, :]])

            with tc.tile_pool(name="gpool1", bufs=8) as gpool1:
                for gi, blks in enumerate(groups):
                    a0, na = gArange[gi]
                    b0, nb = gBrange[gi]
                    gA = gather(gpool1, xA_d, IN_CH, a0, na, "1a")
                    gB = gather(gpool1, xB_d, IN_CH, b0, nb, "1b")
                    for b in blks:
                        ps = aggps.tile([128, IN_CH], FP32, tag="aggps",
                                        name=f"ps1_{b}")
                        for j in range(kA[b]):
                            cg = baseA[b] + j
                            nc.tensor.matmul(ps[:], sel_t[:, cg, :],
                                             gA[:, cg - a0, :], start=(j == 0),
                                             stop=False)
                        for j in range(kB[b]):
                            cg = baseB[b] + j
                            nc.tensor.matmul(ps[:], sel_t[:, cg, :],
                                             gB[:, cg - b0, :], start=False,
                                             stop=(j == kB[b] - 1))
                        aggf = fpool.tile([128, IN_CH], BF16, tag="aggf",
                                          name=f"aggf1_{b}")
                        nc.vector.tensor_scalar_mul(aggf[:], ps[:],
                                                    dinv_t[:, b:b + 1])
                        transform(b, aggf, lambda kc: w1_t[:, kc, :], 2, b1_t,
                                  l1_out)

            # ================= layer 2: phase A (tA), then phase B (tB) =====
            def l2_out(b, mp):
                o = opool.tile([128, OUT_CH], FP32, tag="o", name=f"y{b}")
                nc.scalar.activation(o[:], mp[:], ACT.Copy)
                nc.sync.dma_start(out=y_d[b * 128:(b + 1) * 128, :], in_=o[:])

            aggA = {}
            with tc.tile_pool(name="gpool2", bufs=8) as gpool2:
                groups_eff = groups
                for gi, blks in enumerate(groups_eff):
                    a0, na = gArange[gi]
                    gA = gather(gpool2, tA, HID, a0, na, "2a")
                    for b in blks:
                        ps = aggps.tile([128, HID], FP32, tag="aggps",
                                        name=f"ps2a_{b}")
                        for j in range(kA[b]):
                            cg = baseA[b] + j
                            nc.tensor.matmul(ps[:], sel_t[:, cg, :],
                                             gA[:, cg - a0, :], start=(j == 0),
                                             stop=(j == kA[b] - 1))
                        a = stash.tile([128, HID], BF16, tag="aggA",
                                       name=f"aggA_{b}")
                        aggA[b] = a
                        nc.scalar.activation(a[:], ps[:], ACT.Copy)
                for gi, blks in enumerate(groups_eff):
                    b0, nb = gBrange[gi]
                    gB = gather(gpool2, tB, HID, b0, nb, "2b")
                    for b in blks:
                        ps = aggps.tile([128, HID], FP32, tag="aggps",
                                        name=f"ps2b_{b}")
                        nc.tensor.matmul(ps[:], ident_t[:], aggA[b][:],
                                         start=True, stop=False)
                        for j in range(kB[b]):
                            cg = baseB[b] + j
                            nc.tensor.matmul(ps[:], sel_t[:, cg, :],
                                             gB[:, cg - b0, :], start=False,
                                             stop=(j == kB[b] - 1))
                        aggf = fpool.tile([128, IN_CH], BF16, tag="aggf",
                                          name=f"aggf2_{b}")
                        nc.vector.tensor_scalar_mul(aggf[:, :HID], ps[:],
                                                    dinv_t[:, b:b + 1])
                        transform(b, aggf, lambda kc: w2_t[:], 1, b2_t,
                                  l2_out)

    nc.compile()
    return nc


# ------------------------------------------------------------------- kernel

def kernel(x, edge_index, W1, b1, W2, b2):
    global last_exec_time_ns, last_results
    x = np.asarray(x, dtype=np.float32)
    prep = _prep(x, np.asarray(edge_index))
    nc = _build(prep)

    w1b = np.asarray(W1, dtype=np.float32).astype(NPBF16)
    w2b = np.asarray(W2, dtype=np.float32).astype(NPBF16)
    b1b = np.asarray(b1, dtype=np.float32).reshape(1, -1).astype(NPBF16)
    b2b = np.asarray(b2, dtype=np.float32).reshape(1, -1).astype(NPBF16)
    ident = np.zeros((128, 128), dtype=NPBF16)
    ident[np.arange(128), np.arange(128)] = 1.0

    in_maps = []
    for c in range(N_CORES):
        in_maps.append({
            "xA": prep["xA"], "xB": prep["xB"],
            "sel": prep["sel"][c].reshape(128, -1), "idx": prep["idx_w"][c],
            "dinv": prep["dinv_col"][c],
            "w1": w1b, "w2": w2b, "b1": b1b, "b2": b2b, "ident": ident,
            "ones": np.ones((1, 128), dtype=NPBF16),
        })

    trace = bool(int(os.environ.get("GCN_TRACE", "0")))
    if trace:
        try:
            import ntff_shim
            ntff_shim.install()
        except Exception:
            trace = False
    res = run_bass_kernel_spmd(nc, in_maps, list(range(N_CORES)), trace=trace)
    last_exec_time_ns = res.exec_time_ns
    last_results = res

    y = np.concatenate([np.asarray(res.results[c]["y"]) for c in range(N_CORES)], axis=0)
    return np.ascontiguousarray(y[:N_RAW]).astype(np.float32)


# revision 16
# speedup vs baseline: 1.1105x; 1.0651x over previous
"""2-layer GCN (PyG-style GCNConv) on 8 Trainium2 NeuronCores.

Strategy (v2)
-------------
out = A_hat @ relu(A_hat @ x W1 + b1) @ W2 + b2,  A_hat = D^-1/2 (A+I) D^-1/2.
Aggregate first (A_hat is linear), transform after.

* dinv folding: gather-table rows are pre-scaled by dinv[src]; the aggregated
  PSUM is post-scaled by dinv[dst] at eviction.  Self-loops become ordinary
  edges.  The per-chunk selection matrix is then a pure {0,1} one-hot and is
  stored RESIDENT in SBUF as fp8e4 (exact), loaded once - instead of
  streaming 51MB of bf16 norm matrices from HBM.
* Nodes (padded to 50176) sharded 6272/core; edges partitioned by dst core,
  grouped by (dst-block-of-128, src-table-half) into 128-edge chunks.
  Per chunk one matmul (one-hot lhsT, gathered rows rhs) does scatter+sum.
* Gathers use gpsimd dma_gather with prepare_only+trigger_dma so descriptor
  generation pipelines with the SDMA transfers; calls cover GROUP_BLKS dst
  blocks on rotating SWDGE queues.
* Layer 1 interleaves A/B table chunks in one PSUM accumulation chain.
  Layer 2 runs phase A (table tA, available right after the first
  half-AllGather of h) across all blocks, stashing partial sums in SBUF,
  then phase B once tB lands - overlapping gather work with the collective.
"""

import os
import sys

sys.path.insert(0, "/opt/trn_rl_repo")

import numpy as np
import ml_dtypes

import concourse.bacc as bacc
import concourse.bass as bass
import concourse.mybir as mybir
from concourse.bass_utils import run_bass_kernel_spmd
from concourse.tile import TileContext
from concourse.library_config import mlp

BF16 = mybir.dt.bfloat16
FP32 = mybir.dt.float32
FP8 = mybir.dt.float8e4
I16 = mybir.dt.int16
NPBF16 = ml_dtypes.bfloat16
NPFP8 = ml_dtypes.float8_e4m3

N_CORES = 8
N_RAW = 50000
SHARD = 6272                      # nodes per core (50176 total, padded)
N_PAD = SHARD * N_CORES
NBLK = SHARD // 128               # 49 dst blocks per core
HALF_A = 3200                     # shard rows [0, 3200) -> table A
HALF_B = SHARD - HALF_A           # shard rows [3200, 6272) -> table B
NBLK_A = HALF_A // 128            # 25
NBLK_B = NBLK - NBLK_A            # 24
IN_CH = 256
HID = 128
OUT_CH = 128
GROUP_BLKS = 1                    # dst blocks per gather call

# t2-table slices (within-half block ranges), AllGathered incrementally
SLICES_A = [(0, 9), (9, 8), (17, 8)]
SLICES_B = [(0, 8), (8, 8), (16, 8)]
SROW_A = [0, 9 * 1024, 17 * 1024]
SROW_B = [0, 8 * 1024, 16 * 1024]
ALL_SLICES = SLICES_A + SLICES_B

last_exec_time_ns = None
last_results = None


# ---------------------------------------------------------------- host prep

def _prep(x, edge_index):
    src = np.asarray(edge_index[0], dtype=np.int64)
    dst = np.asarray(edge_index[1], dtype=np.int64)

    deg = np.bincount(dst, minlength=N_PAD).astype(np.float64) + 1.0
    dinv64 = 1.0 / np.sqrt(deg)
    dinv = dinv64.astype(np.float32)

    # self-loops as ordinary edges (for every padded node)
    loop = np.arange(N_PAD, dtype=np.int64)
    src = np.concatenate([src, loop])
    dst = np.concatenate([dst, loop])

    core = dst // SHARD
    blk = (dst % SHARD) // 128
    soff = src % SHARD
    half = (soff >= HALF_A).astype(np.int64)          # 0 = A, 1 = B
    srank = src // SHARD
    dst_off = (dst % 128).astype(np.int64)

    # slice-major table row: row = srow[s] + rank*(128*nb) + (soff%128)*nb
    #                              + (block_in_half - b0)
    bi_half = np.where(half == 0, soff // 128, (soff - HALF_A) // 128)
    srow = np.zeros(src.shape[0], dtype=np.int64)
    nb_arr = np.zeros(src.shape[0], dtype=np.int64)
    b0_arr = np.zeros(src.shape[0], dtype=np.int64)
    for h, slices, srows in ((0, SLICES_A, SROW_A), (1, SLICES_B, SROW_B)):
        for s, (b0, nb) in enumerate(slices):
            m = (half == h) & (bi_half >= b0) & (bi_half < b0 + nb)
            srow[m] = srows[s]
            nb_arr[m] = nb
            b0_arr[m] = b0
    tbl_idx = (srow + srank * (128 * nb_arr) + (soff % 128) * nb_arr
               + (bi_half - b0_arr)).astype(np.int16)

    # chunk counts per (block, half): max over cores (SPMD shared layout)
    gid = core * (2 * NBLK) + half * NBLK + blk
    counts = np.bincount(gid, minlength=N_CORES * 2 * NBLK).reshape(N_CORES, 2, NBLK)
    kA = np.maximum(1, np.ceil(counts[:, 0, :].max(axis=0) / 128).astype(np.int64))
    kB = np.maximum(1, np.ceil(counts[:, 1, :].max(axis=0) / 128).astype(np.int64))

    # global chunk layout: per group g of GROUP_BLKS blocks:
    #   [A-chunks of blocks in g][B-chunks of blocks in g]
    groups = [list(range(g, min(g + GROUP_BLKS, NBLK)))
              for g in range(0, NBLK, GROUP_BLKS)]
    baseA = np.zeros(NBLK, dtype=np.int64)
    baseB = np.zeros(NBLK, dtype=np.int64)
    gArange = []                                       # (chunk0, nchunks) per group
    gBrange = []
    c = 0
    for blks in groups:
        a0 = c
        for b in blks:
            baseA[b] = c
            c += int(kA[b])
        gArange.append((a0, c - a0))
        b0 = c
        for b in blks:
            baseB[b] = c
            c += int(kB[b])
        gBrange.append((b0, c - b0))
    C = c                                              # total chunks per layer
    S = C * 128

    # per-edge slot
    cb = np.where(half == 0, baseA[blk], baseB[blk])
    order = np.lexsort((dst, half, blk, core))
    gsort = gid[order]
    first = np.concatenate([[True], gsort[1:] != gsort[:-1]])
    grp_start = np.flatnonzero(first)
    within = np.arange(order.size) - np.repeat(
        grp_start, np.diff(np.concatenate([grp_start, [order.size]])))
    pos = np.empty_like(order)
    pos[order] = within
    slot = cb * 128 + pos                              # core-local slot id

    # sel: fp8 one-hot, stored K-major: sel[core, k, c, m], slot = c*128 + k
    sel = np.zeros((N_CORES, 128, C, 128), dtype=NPFP8)
    flat = core * (128 * C * 128) + (slot % 128) * (C * 128) + (slot // 128) * 128 + dst_off
    sel.reshape(-1)[flat] = NPFP8(1.0)
    idx16 = np.zeros((N_CORES, S), dtype=np.int16)
    idx16.reshape(-1)[core * S + slot] = tbl_idx

    # wrap idxs: slot j -> partition j%16, col j//16; replicate to 128 partitions
    idx_w = idx16.reshape(N_CORES, S // 16, 16).transpose(0, 2, 1)
    idx_w = np.ascontiguousarray(idx_w)
    idx_w = np.tile(idx_w, (1, 8, 1))                  # [cores, 128, S/16]

    # gather tables: x rows pre-scaled by dinv, K-major within each rank half
    xp = np.zeros((N_PAD, IN_CH), dtype=np.float32)
    xp[:N_RAW] = x
    xp *= dinv[:, None]
    xp = xp.astype(NPFP8)
    xr = xp.reshape(N_CORES, SHARD, IN_CH)

    hA = xr[:, :HALF_A].reshape(N_CORES, NBLK_A, 128, IN_CH)
    hB = xr[:, HALF_A:].reshape(N_CORES, NBLK_B, 128, IN_CH)
    xA = np.ascontiguousarray(np.concatenate(
        [hA[:, b0:b0 + nb].transpose(0, 2, 1, 3).reshape(N_CORES * 128 * nb, IN_CH)
         for (b0, nb) in SLICES_A], axis=0))
    xB = np.ascontiguousarray(np.concatenate(
        [hB[:, b0:b0 + nb].transpose(0, 2, 1, 3).reshape(N_CORES * 128 * nb, IN_CH)
         for (b0, nb) in SLICES_B], axis=0))

    # per-core dinv columns: dinv_col[core][p, b] = dinv[core*SHARD + b*128 + p]
    dinv_col = np.ascontiguousarray(
        dinv.reshape(N_CORES, NBLK, 128).transpose(0, 2, 1))  # [cores, 128, NBLK]

    kAl = [int(v) for v in kA]
    kBl = [int(v) for v in kB]
    return dict(kA=kAl, kB=kBl, baseA=[int(v) for v in baseA],
                baseB=[int(v) for v in baseB], gArange=gArange, gBrange=gBrange,
                groups=groups, C=C, sel=sel, idx_w=idx_w, xA=xA, xB=xB,
                dinv_col=dinv_col)


# ----------------------------------------------------------- device program

def _build(prep):
    kA, kB = prep["kA"], prep["kB"]
    baseA, baseB = prep["baseA"], prep["baseB"]
    gArange, gBrange = prep["gArange"], prep["gBrange"]
    groups = prep["groups"]
    C = prep["C"]
    S = C * 128
    GCH = max(max(n for _, n in gArange), max(n for _, n in gBrange))

    nc = bacc.Bacc("TRN2", target_bir_lowering=False, num_devices=N_CORES,
                   num_swdge_queues=4)

    xA_d = nc.dram_tensor("xA", [N_CORES * HALF_A, IN_CH], FP8, kind="ExternalInput")
    xB_d = nc.dram_tensor("xB", [N_CORES * HALF_B, IN_CH], FP8, kind="ExternalInput")
    sel_d = nc.dram_tensor("sel", [128, C * 128], FP8, kind="ExternalInput")
    idx_d = nc.dram_tensor("idx", [128, S // 16], I16, kind="ExternalInput")
    dinv_d = nc.dram_tensor("dinv", [128, NBLK], FP32, kind="ExternalInput")
    w1_d = nc.dram_tensor("w1", [IN_CH, HID], BF16, kind="ExternalInput")
    w2_d = nc.dram_tensor("w2", [HID, OUT_CH], BF16, kind="ExternalInput")
    b1_d = nc.dram_tensor("b1", [1, HID], BF16, kind="ExternalInput")
    b2_d = nc.dram_tensor("b2", [1, OUT_CH], BF16, kind="ExternalInput")
    ident_d = nc.dram_tensor("ident", [128, 128], BF16, kind="ExternalInput")
    ones_d = nc.dram_tensor("ones", [1, 128], BF16, kind="ExternalInput")
    y_d = nc.dram_tensor("y", [SHARD, OUT_CH], FP32, kind="ExternalOutput")

    bnc2 = [nc.dram_tensor(f"bnc2_{s}", [128, nb * 128], BF16)
            for s, (b0, nb) in enumerate(ALL_SLICES)]
    tA = nc.dram_tensor("tA", [N_CORES * HALF_A, HID], BF16, addr_space="Shared")
    tB = nc.dram_tensor("tB", [N_CORES * HALF_B, HID], BF16, addr_space="Shared")

    def slice_rows(s):
        if s < 3:
            b0, nb = SLICES_A[s]
            return 0, SROW_A[s], 8 * 128 * nb
        b0, nb = SLICES_B[s - 3]
        return 1, SROW_B[s - 3], 8 * 128 * nb

    slice_end = {}
    for s, (b0, nb) in enumerate(ALL_SLICES):
        babs0 = b0 + (0 if s < 3 else NBLK_A)
        slice_end[babs0 + nb - 1] = (s, babs0, nb)

    RG = [list(range(N_CORES))]
    ACT = mybir.ActivationFunctionType

    with TileContext(nc) as tc:
        nc.gpsimd.load_library(mlp)
        import contextlib
        st = contextlib.ExitStack()
        with st:
            consts = st.enter_context(tc.tile_pool(name="consts", bufs=1))
            fpool = st.enter_context(tc.tile_pool(name="fpool", bufs=4))
            ftpool = st.enter_context(tc.tile_pool(name="ftpool", bufs=4))
            opool = st.enter_context(tc.tile_pool(name="opool", bufs=4))
            stash = st.enter_context(tc.tile_pool(name="stash", bufs=NBLK))
            aggps = st.enter_context(tc.tile_pool(name="aggps", bufs=4, space="PSUM"))
            tps = st.enter_context(tc.tile_pool(name="tps", bufs=2, space="PSUM"))
            mmps = st.enter_context(tc.tile_pool(name="mmps", bufs=2, space="PSUM"))

            # ---- constants / resident tensors
            idx_t = consts.tile([128, S // 16], I16)
            nc.sync.dma_start(out=idx_t[:], in_=idx_d[:])
            sel_t = consts.tile([128, C, 128], FP8)
            NSEL = 4
            selsz = [(C // NSEL + (1 if i < C % NSEL else 0)) for i in range(NSEL)]
            off = 0
            for i, sz in enumerate(selsz):
                nc.sync.dma_start(out=sel_t[:, off:off + sz, :],
                                  in_=sel_d[:, off * 128:(off + sz) * 128])
                off += sz
            dinv_t = consts.tile([128, NBLK], FP32)
            nc.sync.dma_start(out=dinv_t[:], in_=dinv_d[:])
            w1_t = consts.tile([128, 2, HID], BF16)
            nc.sync.dma_start(out=w1_t[:], in_=w1_d.rearrange("(c k) m -> k c m", k=128))
            w2_t = consts.tile([128, OUT_CH], BF16)
            nc.sync.dma_start(out=w2_t[:], in_=w2_d[:])
            b1_t = consts.tile([1, HID], BF16)
            nc.sync.dma_start(out=b1_t[:], in_=b1_d[:])
            b2_t = consts.tile([1, OUT_CH], BF16)
            nc.sync.dma_start(out=b2_t[:], in_=b2_d[:])
            ones_t = consts.tile([1, 128], BF16)
            nc.sync.dma_start(out=ones_t[:], in_=ones_d[:])
            ident_t = consts.tile([128, 128], BF16)
            nc.sync.dma_start(out=ident_t[:], in_=ident_d[:])
            h2_t = consts.tile([128, NBLK, HID], BF16)

            qctr = [0]

            def gather(pool, tbl, tbl_ch, c0, n, layer_tag, dt=BF16):
                """dma_gather of chunks [c0, c0+n) on a rotating queue."""
                assert n * 128 <= 2048, n
                q = qctr[0] % 4
                qctr[0] += 1
                g = pool.tile([128, GCH, tbl_ch], dt, tag="g",
                              name=f"g{layer_tag}_{c0}")
                nc.gpsimd.dma_gather(
                    g[:, :n, :], tbl[:], idx_t[:, c0 * 8:(c0 + n) * 8],
                    n * 128, n * 128, tbl_ch, queue_num=q,
                    single_packet=(n * 128 <= 1024))
                return g

            def transform(b, aggf, Wt, nW, bias_t, out_cb):
                mp = mmps.tile([128, 128], FP32, tag="mmps", name=f"mm{id(aggf)}_{b}")
                for kc in range(nW):
                    tp = tps.tile([128, 128], BF16, tag="tp", name=f"tp{id(aggf)}_{b}_{kc}")
                    nc.tensor.transpose(tp[:], aggf[:, kc * 128:(kc + 1) * 128],
                                        ident_t[:])
                    ft = ftpool.tile([128, 128], BF16, tag="fT", name=f"fT{id(aggf)}_{b}_{kc}")
                    nc.scalar.activation(ft[:], tp[:], ACT.Copy)
                    nc.tensor.matmul(mp[:], ft[:], Wt(kc), start=(kc == 0), stop=False)
                nc.tensor.matmul(mp[:], ones_t[:], bias_t[:], start=False, stop=True)
                out_cb(b, mp)

            # ================= layer 1: single phase, A/B interleaved =======
            def l1_out(b, mp):
                # h2 table row = relu(z) * dinv[dst]  (== relu(z*dinv), dinv>0)
                nc.scalar.activation(h2_t[:, b, :], mp[:], ACT.Relu,
                                     scale=dinv_t[:, b:b + 1])
                if b in slice_end:
                    s, babs0, nb = slice_end[b]
                    nc.scalar.dma_start(out=bnc2[s][:],
                                        in_=h2_t[:, babs0:babs0 + nb, :])
                    half, r0, nr = slice_rows(s)
                    tbl = tA if half == 0 else tB
                    nc.gpsimd.collective_compute(
                        "AllGather", mybir.AluOpType.bypass, replica_groups=RG,
                        ins=[bnc2[s][:]], outs=[tbl[r0:r0 + nr, :]])

            with tc.tile_pool(name="gpool1", bufs=8) as gpool1:
                for gi, blks in enumerate(groups):
                    a0, na = gArange[gi]
                    b0, nb = gBrange[gi]
                    gA = gather(gpool1, xA_d, IN_CH, a0, na, "1a", dt=FP8)
                    gB = gather(gpool1, xB_d, IN_CH, b0, nb, "1b", dt=FP8)
                    for b in blks:
                        ps = aggps.tile([128, IN_CH], FP32, tag="aggps",
                                        name=f"ps1_{b}")
                        for j in range(kA[b]):
                            cg = baseA[b] + j
                            nc.tensor.matmul(ps[:], sel_t[:, cg, :],
                                             gA[:, cg - a0, :], start=(j == 0),
                                             stop=False)
                        for j in range(kB[b]):
                            cg = baseB[b] + j
                            nc.tensor.matmul(ps[:], sel_t[:, cg, :],
                                             gB[:, cg - b0, :], start=False,
                                             stop=(j == kB[b] - 1))
                        aggf = fpool.tile([128, IN_CH], BF16, tag="aggf",
                                          name=f"aggf1_{b}")
                        nc.vector.tensor_scalar_mul(aggf[:], ps[:],
                                                    dinv_t[:, b:b + 1])
                        transform(b, aggf, lambda kc: w1_t[:, kc, :], 2, b1_t,
                                  l1_out)

            # ================= layer 2: phase A (tA), then phase B (tB) =====
            def l2_out(b, mp):
                o = opool.tile([128, OUT_CH], FP32, tag="o", name=f"y{b}")
                nc.scalar.activation(o[:], mp[:], ACT.Copy)
                nc.sync.dma_start(out=y_d[b * 128:(b + 1) * 128, :], in_=o[:])

            aggA = {}
            with tc.tile_pool(name="gpool2", bufs=8) as gpool2:
                groups_eff = groups
                for gi, blks in enumerate(groups_eff):
                    a0, na = gArange[gi]
                    gA = gather(gpool2, tA, HID, a0, na, "2a")
                    for b in blks:
                        ps = aggps.tile([128, HID], FP32, tag="aggps",
                                        name=f"ps2a_{b}")
                        for j in range(kA[b]):
                            cg = baseA[b] + j
                            nc.tensor.matmul(ps[:], sel_t[:, cg, :],
                                             gA[:, cg - a0, :], start=(j == 0),
                                             stop=(j == kA[b] - 1))
                        a = stash.tile([128, HID], BF16, tag="aggA",
                                       name=f"aggA_{b}")
                        aggA[b] = a
                        nc.scalar.activation(a[:], ps[:], ACT.Copy)
                for gi, blks in enumerate(groups_eff):
                    b0, nb = gBrange[gi]
                    gB = gather(gpool2, tB, HID, b0, nb, "2b")
                    for b in blks:
                        ps = aggps.tile([128, HID], FP32, tag="aggps",
                                        name=f"ps2b_{b}")
                        nc.tensor.matmul(ps[:], ident_t[:], aggA[b][:],
                                         start=True, stop=False)
                        for j in range(kB[b]):
                            cg = baseB[b] + j
                            nc.tensor.matmul(ps[:], sel_t[:, cg, :],
                                             gB[:, cg - b0, :], start=False,
                                             stop=(j == kB[b] - 1))
                        aggf = fpool.tile([128, IN_CH], BF16, tag="aggf",
                                          name=f"aggf2_{b}")
                        nc.vector.tensor_scalar_mul(aggf[:, :HID], ps[:],
                                                    dinv_t[:, b:b + 1])
                        transform(b, aggf, lambda kc: w2_t[:], 1, b2_t,
                                  l2_out)

    nc.compile()
    return nc


# ------------------------------------------------------------------- kernel

def kernel(x, edge_index, W1, b1, W2, b2):
    global last_exec_time_ns, last_results
    x = np.asarray(x, dtype=np.float32)
    prep = _prep(x, np.asarray(edge_index))
    nc = _build(prep)

    w1b = np.asarray(W1, dtype=np.float32).astype(NPBF16)
    w2b = np.asarray(W2, dtype=np.float32).astype(NPBF16)
    b1b = np.asarray(b1, dtype=np.float32).reshape(1, -1).astype(NPBF16)
    b2b = np.asarray(b2, dtype=np.float32).reshape(1, -1).astype(NPBF16)
    ident = np.zeros((128, 128), dtype=NPBF16)
    ident[np.arange(128), np.arange(128)] = 1.0

    in_maps = []
    for c in range(N_CORES):
        in_maps.append({
            "xA": prep["xA"], "xB": prep["xB"],
            "sel": prep["sel"][c].reshape(128, -1), "idx": prep["idx_w"][c],
            "dinv": prep["dinv_col"][c],
            "w1": w1b, "w2": w2b, "b1": b1b, "b2": b2b, "ident": ident,
            "ones": np.ones((1, 128), dtype=NPBF16),
        })

    trace = bool(int(os.environ.get("GCN_TRACE", "0")))
    if trace:
        try:
            import ntff_shim
            ntff_shim.install()
        except Exception:
            trace = False
    res = run_bass_kernel_spmd(nc, in_maps, list(range(N_CORES)), trace=trace)
    last_exec_time_ns = res.exec_time_ns
    last_results = res

    y = np.concatenate([np.asarray(res.results[c]["y"]) for c in range(N_CORES)], axis=0)
    return np.ascontiguousarray(y[:N_RAW]).astype(np.float32)


# revision 18
# speedup vs baseline: 1.1587x; 1.0434x over previous
"""2-layer GCN (PyG-style GCNConv) on 8 Trainium2 NeuronCores.

Strategy (v2)
-------------
out = A_hat @ relu(A_hat @ x W1 + b1) @ W2 + b2,  A_hat = D^-1/2 (A+I) D^-1/2.
Aggregate first (A_hat is linear), transform after.

* dinv folding: gather-table rows are pre-scaled by dinv[src]; the aggregated
  PSUM is post-scaled by dinv[dst] at eviction.  Self-loops become ordinary
  edges.  The per-chunk selection matrix is then a pure {0,1} one-hot and is
  stored RESIDENT in SBUF as fp8e4 (exact), loaded once - instead of
  streaming 51MB of bf16 norm matrices from HBM.
* Nodes (padded to 50176) sharded 6272/core; edges partitioned by dst core,
  grouped by (dst-block-of-128, src-table-half) into 128-edge chunks.
  Per chunk one matmul (one-hot lhsT, gathered rows rhs) does scatter+sum.
* Gathers use gpsimd dma_gather with prepare_only+trigger_dma so descriptor
  generation pipelines with the SDMA transfers; calls cover GROUP_BLKS dst
  blocks on rotating SWDGE queues.
* Layer 1 interleaves A/B table chunks in one PSUM accumulation chain.
  Layer 2 runs phase A (table tA, available right after the first
  half-AllGather of h) across all blocks, stashing partial sums in SBUF,
  then phase B once tB lands - overlapping gather work with the collective.
"""

import os
import sys

sys.path.insert(0, "/opt/trn_rl_repo")

import numpy as np
import ml_dtypes

import concourse.bacc as bacc
import concourse.bass as bass
import concourse.mybir as mybir
from concourse.bass_utils import run_bass_kernel_spmd
from concourse.tile import TileContext
from concourse.library_config import mlp

BF16 = mybir.dt.bfloat16
FP32 = mybir.dt.float32
FP8 = mybir.dt.float8e4
I16 = mybir.dt.int16
NPBF16 = ml_dtypes.bfloat16
NPFP8 = ml_dtypes.float8_e4m3

N_CORES = 8
N_RAW = 50000
SHARD = 6272                      # nodes per core (50176 total, padded)
N_PAD = SHARD * N_CORES
NBLK = SHARD // 128               # 49 dst blocks per core
HALF_A = 3200                     # shard rows [0, 3200) -> table A
HALF_B = SHARD - HALF_A           # shard rows [3200, 6272) -> table B
NBLK_A = HALF_A // 128            # 25
NBLK_B = NBLK - NBLK_A            # 24
IN_CH = 256
HID = 128
OUT_CH = 128
GROUP_BLKS = 1                    # dst blocks per gather call

# t2-table slices (within-half block ranges), AllGathered incrementally
SLICES_A = [(0, 9), (9, 8), (17, 8)]
SLICES_B = [(0, 8), (8, 8), (16, 8)]
SROW_A = [0, 9 * 1024, 17 * 1024]
SROW_B = [0, 8 * 1024, 16 * 1024]
ALL_SLICES = SLICES_A + SLICES_B

last_exec_time_ns = None
last_results = None


# ---------------------------------------------------------------- host prep

def _prep(x, edge_index):
    src = np.asarray(edge_index[0], dtype=np.int64)
    dst = np.asarray(edge_index[1], dtype=np.int64)

    deg = np.bincount(dst, minlength=N_PAD).astype(np.float64) + 1.0
    dinv64 = 1.0 / np.sqrt(deg)
    dinv = dinv64.astype(np.float32)

    # self-loops as ordinary edges (for every padded node)
    loop = np.arange(N_PAD, dtype=np.int64)
    src = np.concatenate([src, loop])
    dst = np.concatenate([dst, loop])

    core = dst // SHARD
    blk = (dst % SHARD) // 128
    soff = src % SHARD
    half = (soff >= HALF_A).astype(np.int64)          # 0 = A, 1 = B
    srank = src // SHARD
    dst_off = (dst % 128).astype(np.int64)

    # slice-major table row: row = srow[s] + rank*(128*nb) + (soff%128)*nb
    #                              + (block_in_half - b0)
    bi_half = np.where(half == 0, soff // 128, (soff - HALF_A) // 128)
    srow = np.zeros(src.shape[0], dtype=np.int64)
    nb_arr = np.zeros(src.shape[0], dtype=np.int64)
    b0_arr = np.zeros(src.shape[0], dtype=np.int64)
    for h, slices, srows in ((0, SLICES_A, SROW_A), (1, SLICES_B, SROW_B)):
        for s, (b0, nb) in enumerate(slices):
            m = (half == h) & (bi_half >= b0) & (bi_half < b0 + nb)
            srow[m] = srows[s]
            nb_arr[m] = nb
            b0_arr[m] = b0
    tbl_idx = (srow + srank * (128 * nb_arr) + (soff % 128) * nb_arr
               + (bi_half - b0_arr)).astype(np.int16)

    # chunk counts per (block, half): max over cores (SPMD shared layout)
    gid = core * (2 * NBLK) + half * NBLK + blk
    counts = np.bincount(gid, minlength=N_CORES * 2 * NBLK).reshape(N_CORES, 2, NBLK)
    kA = np.maximum(1, np.ceil(counts[:, 0, :].max(axis=0) / 128).astype(np.int64))
    kB = np.maximum(1, np.ceil(counts[:, 1, :].max(axis=0) / 128).astype(np.int64))

    # global chunk layout: per group g of GROUP_BLKS blocks:
    #   [A-chunks of blocks in g][B-chunks of blocks in g]
    groups = [list(range(g, min(g + GROUP_BLKS, NBLK)))
              for g in range(0, NBLK, GROUP_BLKS)]
    baseA = np.zeros(NBLK, dtype=np.int64)
    baseB = np.zeros(NBLK, dtype=np.int64)
    gArange = []                                       # (chunk0, nchunks) per group
    gBrange = []
    c = 0
    for blks in groups:
        a0 = c
        for b in blks:
            baseA[b] = c
            c += int(kA[b])
        gArange.append((a0, c - a0))
        b0 = c
        for b in blks:
            baseB[b] = c
            c += int(kB[b])
        gBrange.append((b0, c - b0))
    C = c                                              # total chunks per layer
    S = C * 128

    # per-edge slot
    cb = np.where(half == 0, baseA[blk], baseB[blk])
    order = np.lexsort((dst, half, blk, core))
    gsort = gid[order]
    first = np.concatenate([[True], gsort[1:] != gsort[:-1]])
    grp_start = np.flatnonzero(first)
    within = np.arange(order.size) - np.repeat(
        grp_start, np.diff(np.concatenate([grp_start, [order.size]])))
    pos = np.empty_like(order)
    pos[order] = within
    slot = cb * 128 + pos                              # core-local slot id

    # sel: fp8 one-hot, stored K-major: sel[core, k, c, m], slot = c*128 + k
    sel = np.zeros((N_CORES, 128, C, 128), dtype=NPFP8)
    flat = core * (128 * C * 128) + (slot % 128) * (C * 128) + (slot // 128) * 128 + dst_off
    sel.reshape(-1)[flat] = NPFP8(1.0)
    idx16 = np.zeros((N_CORES, S), dtype=np.int16)
    idx16.reshape(-1)[core * S + slot] = tbl_idx

    # wrap idxs: slot j -> partition j%16, col j//16; replicate to 128 partitions
    idx_w = idx16.reshape(N_CORES, S // 16, 16).transpose(0, 2, 1)
    idx_w = np.ascontiguousarray(idx_w)
    idx_w = np.tile(idx_w, (1, 8, 1))                  # [cores, 128, S/16]

    # gather tables: x rows pre-scaled by dinv, K-major within each rank half
    xp = np.zeros((N_PAD, IN_CH), dtype=np.float32)
    xp[:N_RAW] = x
    xp *= dinv[:, None]
    xp = xp.astype(NPFP8)
    xr = xp.reshape(N_CORES, SHARD, IN_CH)

    hA = xr[:, :HALF_A].reshape(N_CORES, NBLK_A, 128, IN_CH)
    hB = xr[:, HALF_A:].reshape(N_CORES, NBLK_B, 128, IN_CH)
    xA = np.ascontiguousarray(np.concatenate(
        [hA[:, b0:b0 + nb].transpose(0, 2, 1, 3).reshape(N_CORES * 128 * nb, IN_CH)
         for (b0, nb) in SLICES_A], axis=0))
    xB = np.ascontiguousarray(np.concatenate(
        [hB[:, b0:b0 + nb].transpose(0, 2, 1, 3).reshape(N_CORES * 128 * nb, IN_CH)
         for (b0, nb) in SLICES_B], axis=0))

    # per-core dinv columns: dinv_col[core][p, b] = dinv[core*SHARD + b*128 + p]
    dinv_col = np.ascontiguousarray(
        dinv.reshape(N_CORES, NBLK, 128).transpose(0, 2, 1))  # [cores, 128, NBLK]

    kAl = [int(v) for v in kA]
    kBl = [int(v) for v in kB]
    return dict(kA=kAl, kB=kBl, baseA=[int(v) for v in baseA],
                baseB=[int(v) for v in baseB], gArange=gArange, gBrange=gBrange,
                groups=groups, C=C, sel=sel, idx_w=idx_w, xA=xA, xB=xB,
                dinv_col=dinv_col)


# ----------------------------------------------------------- device program

def _build(prep):
    kA, kB = prep["kA"], prep["kB"]
    baseA, baseB = prep["baseA"], prep["baseB"]
    gArange, gBrange = prep["gArange"], prep["gBrange"]
    groups = prep["groups"]
    C = prep["C"]
    S = C * 128
    GCH = max(max(n for _, n in gArange), max(n for _, n in gBrange))

    nc = bacc.Bacc("TRN2", target_bir_lowering=False, num_devices=N_CORES,
                   num_swdge_queues=4)

    xA_d = nc.dram_tensor("xA", [N_CORES * HALF_A, IN_CH], FP8, kind="ExternalInput")
    xB_d = nc.dram_tensor("xB", [N_CORES * HALF_B, IN_CH], FP8, kind="ExternalInput")
    sel_d = nc.dram_tensor("sel", [128, C * 128], FP8, kind="ExternalInput")
    idx_d = nc.dram_tensor("idx", [128, S // 16], I16, kind="ExternalInput")
    dinv_d = nc.dram_tensor("dinv", [128, NBLK], FP32, kind="ExternalInput")
    w1_d = nc.dram_tensor("w1", [IN_CH, HID], BF16, kind="ExternalInput")
    w2_d = nc.dram_tensor("w2", [HID, OUT_CH], BF16, kind="ExternalInput")
    b1_d = nc.dram_tensor("b1", [1, HID], BF16, kind="ExternalInput")
    b2_d = nc.dram_tensor("b2", [1, OUT_CH], BF16, kind="ExternalInput")
    ident_d = nc.dram_tensor("ident", [128, 128], BF16, kind="ExternalInput")
    ones_d = nc.dram_tensor("ones", [1, 128], BF16, kind="ExternalInput")
    y_d = nc.dram_tensor("y", [SHARD, OUT_CH], FP32, kind="ExternalOutput")

    bnc2 = [nc.dram_tensor(f"bnc2_{s}", [128, nb * 128], BF16)
            for s, (b0, nb) in enumerate(ALL_SLICES)]
    tA = nc.dram_tensor("tA", [N_CORES * HALF_A, HID], BF16, addr_space="Shared")
    tB = nc.dram_tensor("tB", [N_CORES * HALF_B, HID], BF16, addr_space="Shared")

    def slice_rows(s):
        if s < 3:
            b0, nb = SLICES_A[s]
            return 0, SROW_A[s], 8 * 128 * nb
        b0, nb = SLICES_B[s - 3]
        return 1, SROW_B[s - 3], 8 * 128 * nb

    slice_end = {}
    for s, (b0, nb) in enumerate(ALL_SLICES):
        babs0 = b0 + (0 if s < 3 else NBLK_A)
        slice_end[babs0 + nb - 1] = (s, babs0, nb)

    RG = [list(range(N_CORES))]
    ACT = mybir.ActivationFunctionType

    with TileContext(nc) as tc:
        nc.gpsimd.load_library(mlp)
        import contextlib
        st = contextlib.ExitStack()
        with st:
            consts = st.enter_context(tc.tile_pool(name="consts", bufs=1))
            fpool = st.enter_context(tc.tile_pool(name="fpool", bufs=4))
            ftpool = st.enter_context(tc.tile_pool(name="ftpool", bufs=4))
            opool = st.enter_context(tc.tile_pool(name="opool", bufs=4))
            stash = st.enter_context(tc.tile_pool(name="stash", bufs=NBLK))
            aggps = st.enter_context(tc.tile_pool(name="aggps", bufs=4, space="PSUM"))
            tps = st.enter_context(tc.tile_pool(name="tps", bufs=2, space="PSUM"))
            mmps = st.enter_context(tc.tile_pool(name="mmps", bufs=2, space="PSUM"))

            # ---- constants / resident tensors
            idx_t = consts.tile([128, S // 16], I16)
            nc.sync.dma_start(out=idx_t[:], in_=idx_d[:])
            sel_t = consts.tile([128, C, 128], FP8)
            NSEL = 4
            selsz = [(C // NSEL + (1 if i < C % NSEL else 0)) for i in range(NSEL)]
            off = 0
            for i, sz in enumerate(selsz):
                nc.sync.dma_start(out=sel_t[:, off:off + sz, :],
                                  in_=sel_d[:, off * 128:(off + sz) * 128])
                off += sz
            dinv_t = consts.tile([128, NBLK], FP32)
            nc.sync.dma_start(out=dinv_t[:], in_=dinv_d[:])
            w1_t = consts.tile([128, 2, HID], BF16)
            nc.sync.dma_start(out=w1_t[:], in_=w1_d.rearrange("(c k) m -> k c m", k=128))
            w2_t = consts.tile([128, OUT_CH], BF16)
            nc.sync.dma_start(out=w2_t[:], in_=w2_d[:])
            b1_t = consts.tile([1, HID], BF16)
            nc.sync.dma_start(out=b1_t[:], in_=b1_d[:])
            b2_t = consts.tile([1, OUT_CH], BF16)
            nc.sync.dma_start(out=b2_t[:], in_=b2_d[:])
            ones_t = consts.tile([1, 128], BF16)
            nc.sync.dma_start(out=ones_t[:], in_=ones_d[:])
            ident_t = consts.tile([128, 128], BF16)
            nc.sync.dma_start(out=ident_t[:], in_=ident_d[:])
            h2_t = consts.tile([128, NBLK, HID], BF16)

            qctr = [0]

            def gather(pool, tbl, tbl_ch, c0, n, layer_tag, dt=BF16):
                """dma_gather of chunks [c0, c0+n) on a rotating queue."""
                assert n * 128 <= 2048, n
                q = qctr[0] % 4
                qctr[0] += 1
                g = pool.tile([128, GCH, tbl_ch], dt, tag="g",
                              name=f"g{layer_tag}_{c0}")
                nc.gpsimd.dma_gather(
                    g[:, :n, :], tbl[:], idx_t[:, c0 * 8:(c0 + n) * 8],
                    n * 128, n * 128, tbl_ch, queue_num=q,
                    single_packet=(n * 128 <= 1024))
                return g

            def transform(b, aggf, Wt, nW, bias_t, out_cb):
                mp = mmps.tile([128, 128], FP32, tag="mmps", name=f"mm{id(aggf)}_{b}")
                for kc in range(nW):
                    tp = tps.tile([128, 128], BF16, tag="tp", name=f"tp{id(aggf)}_{b}_{kc}")
                    nc.tensor.transpose(tp[:], aggf[:, kc * 128:(kc + 1) * 128],
                                        ident_t[:])
                    ft = ftpool.tile([128, 128], BF16, tag="fT", name=f"fT{id(aggf)}_{b}_{kc}")
                    nc.scalar.activation(ft[:], tp[:], ACT.Copy)
                    nc.tensor.matmul(mp[:], ft[:], Wt(kc), start=(kc == 0), stop=False)
                nc.tensor.matmul(mp[:], ones_t[:], bias_t[:], start=False, stop=True)
                out_cb(b, mp)

            # ================= layer 1: single phase, A/B interleaved =======
            def l1_out(b, mp):
                # h2 table row = relu(z) * dinv[dst]  (== relu(z*dinv), dinv>0)
                nc.scalar.activation(h2_t[:, b, :], mp[:], ACT.Relu,
                                     scale=dinv_t[:, b:b + 1])
                if b in slice_end:
                    s, babs0, nb = slice_end[b]
                    nc.scalar.dma_start(out=bnc2[s][:],
                                        in_=h2_t[:, babs0:babs0 + nb, :])
                    half, r0, nr = slice_rows(s)
                    tbl = tA if half == 0 else tB
                    nc.gpsimd.collective_compute(
                        "AllGather", mybir.AluOpType.bypass, replica_groups=RG,
                        ins=[bnc2[s][:]], outs=[tbl[r0:r0 + nr, :]])

            with tc.tile_pool(name="gpool1", bufs=8) as gpool1:
                for gi, blks in enumerate(groups):
                    a0, na = gArange[gi]
                    b0, nb = gBrange[gi]
                    gA = gather(gpool1, xA_d, IN_CH, a0, na, "1a", dt=FP8)
                    gB = gather(gpool1, xB_d, IN_CH, b0, nb, "1b", dt=FP8)
                    for b in blks:
                        ps = aggps.tile([128, IN_CH], FP32, tag="aggps",
                                        name=f"ps1_{b}")
                        for j in range(kA[b]):
                            cg = baseA[b] + j
                            nc.tensor.matmul(ps[:], sel_t[:, cg, :],
                                             gA[:, cg - a0, :], start=(j == 0),
                                             stop=False)
                        for j in range(kB[b]):
                            cg = baseB[b] + j
                            nc.tensor.matmul(ps[:], sel_t[:, cg, :],
                                             gB[:, cg - b0, :], start=False,
                                             stop=(j == kB[b] - 1))
                        aggf = fpool.tile([128, IN_CH], BF16, tag="aggf",
                                          name=f"aggf1_{b}")
                        nc.vector.tensor_scalar_mul(aggf[:], ps[:],
                                                    dinv_t[:, b:b + 1])
                        transform(b, aggf, lambda kc: w1_t[:, kc, :], 2, b1_t,
                                  l1_out)

            # ================= layer 2: phase A (tA), then phase B (tB) =====
            def l2_out(b, mp):
                o = opool.tile([128, OUT_CH], FP32, tag="o", name=f"y{b}")
                nc.scalar.activation(o[:], mp[:], ACT.Copy)
                nc.sync.dma_start(out=y_d[b * 128:(b + 1) * 128, :], in_=o[:])

            aggA = {}
            with tc.tile_pool(name="gpool2", bufs=8) as gpool2:
                groups_eff = groups
                for gi, blks in enumerate(groups_eff):
                    a0, na = gArange[gi]
                    gA = gather(gpool2, tA, HID, a0, na, "2a")
                    for b in blks:
                        ps = aggps.tile([128, HID], FP32, tag="aggps",
                                        name=f"ps2a_{b}")
                        for j in range(kA[b]):
                            cg = baseA[b] + j
                            nc.tensor.matmul(ps[:], sel_t[:, cg, :],
                                             gA[:, cg - a0, :], start=(j == 0),
                                             stop=(j == kA[b] - 1))
                        a = stash.tile([128, HID], BF16, tag="aggA",
                                       name=f"aggA_{b}")
                        aggA[b] = a
                        nc.scalar.activation(a[:], ps[:], ACT.Copy)
                for gi, blks in enumerate(groups_eff):
                    b0, nb = gBrange[gi]
                    gB = gather(gpool2, tB, HID, b0, nb, "2b")
                    for b in blks:
                        ps = aggps.tile([128, HID], FP32, tag="aggps",
                                        name=f"ps2b_{b}")
                        nc.tensor.matmul(ps[:], ident_t[:], aggA[b][:],
                                         start=True, stop=False)
                        for j in range(kB[b]):
                            cg = baseB[b] + j
                            nc.tensor.matmul(ps[:], sel_t[:, cg, :],
                                             gB[:, cg - b0, :], start=False,
                                             stop=(j == kB[b] - 1))
                        aggf = fpool.tile([128, IN_CH], BF16, tag="aggf",
                                          name=f"aggf2_{b}")
                        nc.vector.tensor_scalar_mul(aggf[:, :HID], ps[:],
                                                    dinv_t[:, b:b + 1])
                        transform(b, aggf, lambda kc: w2_t[:], 1, b2_t,
                                  l2_out)

    nc.compile()
    return nc


# ------------------------------------------------------------------- kernel

def kernel(x, edge_index, W1, b1, W2, b2):
    global last_exec_time_ns, last_results
    x = np.asarray(x, dtype=np.float32)
    prep = _prep(x, np.asarray(edge_index))
    nc = _build(prep)

    w1b = np.asarray(W1, dtype=np.float32).astype(NPBF16)
    w2b = np.asarray(W2, dtype=np.float32).astype(NPBF16)
    b1b = np.asarray(b1, dtype=np.float32).reshape(1, -1).astype(NPBF16)
    b2b = np.asarray(b2, dtype=np.float32).reshape(1, -1).astype(NPBF16)
    ident = np.zeros((128, 128), dtype=NPBF16)
    ident[np.arange(128), np.arange(128)] = 1.0

    in_maps = []
    for c in range(N_CORES):
        in_maps.append({
            "xA": prep["xA"], "xB": prep["xB"],
            "sel": prep["sel"][c].reshape(128, -1), "idx": prep["idx_w"][c],
            "dinv": prep["dinv_col"][c],
            "w1": w1b, "w2": w2b, "b1": b1b, "b2": b2b, "ident": ident,
            "ones": np.ones((1, 128), dtype=NPBF16),
        })

    trace = bool(int(os.environ.get("GCN_TRACE", "0")))
    if trace:
        try:
            import ntff_shim
            ntff_shim.install()
        except Exception:
            trace = False
    res = run_bass_kernel_spmd(nc, in_maps, list(range(N_CORES)), trace=trace)
    last_exec_time_ns = res.exec_time_ns
    last_results = res

    y = np.concatenate([np.asarray(res.results[c]["y"]) for c in range(N_CORES)], axis=0)
    return np.ascontiguousarray(y[:N_RAW]).astype(np.float32)
